# revision 6
# baseline (speedup 1.0000x reference)
"""LlamaAttention (GQA, no mask) on 8 Trainium2 NeuronCores.

Sharding: 8 cores = 2 (batch) x 4 (head groups of 8 heads / 2 KV heads).
Per core (bf16 compute, fp32 accumulation):
  qT  = (x_b @ wq_g)^T            [512, 2048]   (head dims on partitions)
  kTd = (x_b @ wk_g)^T duplicated [128, 2, 2048]
  v   = x_b @ wv_g (+ ones col)   [2048, 2, 65]
  per head pair: sT[k,q] matmuls -> exp on ACT -> flipped AV matmuls
    out[q-tile 128, 65] (full-M, half the PE streaming of the [65, q]
    orientation), accumulated in a packed 3-bank PSUM tile; per-partition
    reciprocal + ACT scale-mul normalize; PE transpose (identity matmul)
    restores at^T[d, q] for o_proj.
  out_partial = at @ wo_g         [2048, 2048] fp32
Host sums the 4 head-group partials per batch.
v/q/o projection matmuls are emitted as ~2048-cycle filler quanta inside
the attention kc loop (dedicated 1-bank PSUM buffer) so the PE array keeps
streaming while ACT computes exp.
"""

import numpy as np
import ml_dtypes

S = 2048          # sequence length
D = 2048          # model dim
HD = 64           # head dim
GH = 8            # heads per core
QC = GH * HD      # 512 q cols per core
KVC = 128         # kv cols per core (2 kv heads)
DC = D // 128     # 16 contraction chunks
SC = S // 128     # 16 seq chunks
SCALE = HD ** -0.5

_CACHE = {}


def _build():
    import concourse.bass as bass
    import concourse.mybir as mybir
    import concourse.tile as tile
    from concourse import bacc, masks

    f32 = mybir.dt.float32
    bf16 = mybir.dt.bfloat16
    Exp = mybir.ActivationFunctionType.Exp
    Copy = mybir.ActivationFunctionType.Copy

    nc = bacc.Bacc("TRN2", target_bir_lowering=False, debug=False, num_devices=8)

    xt = nc.dram_tensor("xt", [D, S], bf16, kind="ExternalInput").ap()
    wq = nc.dram_tensor("wq", [D, QC], bf16, kind="ExternalInput").ap()
    wk = nc.dram_tensor("wk", [D, KVC], bf16, kind="ExternalInput").ap()
    wv = nc.dram_tensor("wv", [D, KVC], bf16, kind="ExternalInput").ap()
    wo = nc.dram_tensor("wo", [QC, D], bf16, kind="ExternalInput").ap()
    out = nc.dram_tensor("out", [S, D], f32, kind="ExternalOutput").ap()

    with tile.TileContext(nc) as tc:
        with tc.tile_pool(name="const", bufs=1) as const, \
             tc.tile_pool(name="sps", bufs=2, space="PSUM") as sps, \
             tc.tile_pool(name="pjp", bufs=1, space="PSUM") as pjp, \
             tc.tile_pool(name="oap", bufs=1, space="PSUM") as oap, \
             tc.tile_pool(name="ev", bufs=2) as ev, \
             tc.tile_pool(name="ppool", bufs=3) as ppool, \
             tc.tile_pool(name="dpool", bufs=2) as dpool:

            # resident inputs, partition-chunked layouts (small weights first,
            # xt per-chunk so the k/v projections can start early)
            wk_all = const.tile([128, DC, KVC], bf16, tag="wk_all")
            nc.sync.dma_start(out=wk_all[:], in_=wk.rearrange("(c p) n -> p c n", p=128))
            wv_all = const.tile([128, DC, KVC], bf16, tag="wv_all")
            nc.sync.dma_start(out=wv_all[:], in_=wv.rearrange("(c p) n -> p c n", p=128))
            xt_all = const.tile([128, DC, S], bf16, tag="xt_all")
            xt_re = xt.rearrange("(c p) s -> p c s", p=128)
            for dc in range(DC):
                nc.sync.dma_start(out=xt_all[:, dc, :], in_=xt_re[:, dc, :])
            wq_all = const.tile([128, DC, QC], bf16, tag="wq_all")
            nc.sync.dma_start(out=wq_all[:], in_=wq.rearrange("(c p) n -> p c n", p=128))
            wo_all = const.tile([128, QC // 128, D], bf16, tag="wo_all")
            nc.sync.dma_start(out=wo_all[:], in_=wo.rearrange("(c p) n -> p c n", p=128))

            ident = const.tile([128, 128], bf16, tag="ident")
            masks.make_identity(nc, ident[:])

            # persistent intermediates
            qpair = const.tile([128, 4, S], bf16, tag="qpair")     # q^T
            ktd = const.tile([128, 2, S], bf16, tag="ktd")         # k^T dup per kv head
            vv = const.tile([128, SC, 130], bf16, tag="vv")        # v (+ones cols)
            at = const.tile([128, 4, S], bf16, tag="at")           # attn out^T

            nc.vector.memset(vv[:, :, 64:65], 1.0)
            nc.vector.memset(vv[:, :, 129:130], 1.0)

            def mmacc(out_t, lhsT, rhs, width, start, stop):
                # moving-operand ISA limit is 512: split wide matmuls
                for o in range(0, width, 512):
                    nc.tensor.matmul(out_t[:, o:o + 512], lhsT,
                                     rhs[:, o:o + 512], start=start, stop=stop)

            # ---------------- k projection (needed before any scores) -------
            for nb in range(2):
                ps = sps.tile([128, 1024], f32, tag="s_ps")
                for dc in range(DC):
                    mmacc(ps, wk_all[:, dc, :],
                          xt_all[:, dc, nb * 1024:(nb + 1) * 1024], 1024,
                          (dc == 0), (dc == DC - 1))
                kt_sb = ev.tile([128, 1024], bf16, tag="kt_sb")
                nc.vector.tensor_copy(kt_sb[:], ps[:])
                sl = slice(nb * 1024, (nb + 1) * 1024)
                nc.sync.dma_start(out=ktd[0:64, 0, sl], in_=kt_sb[0:64, :])
                nc.sync.dma_start(out=ktd[64:128, 0, sl], in_=kt_sb[0:64, :])
                nc.sync.dma_start(out=ktd[0:64, 1, sl], in_=kt_sb[64:128, :])
                nc.sync.dma_start(out=ktd[64:128, 1, sl], in_=kt_sb[64:128, :])

            # ---------------- filler work units (~2048 PE cycles each) ------
            def v_chunk(sc):
                def run():
                    ps = pjp.tile([128, 512], f32, tag="pj")
                    for dc in range(DC):
                        nc.tensor.matmul(ps[:, 0:KVC],
                                         xt_all[:, dc, sc * 128:(sc + 1) * 128],
                                         wv_all[:, dc, :],
                                         start=(dc == 0), stop=(dc == DC - 1))
                    nc.vector.tensor_copy(vv[:, sc, 0:64], ps[:, 0:64])
                    nc.vector.tensor_copy(vv[:, sc, 65:129], ps[:, 64:128])
                return run, 2048

            def q_half(qm, jbb, h):
                def run():
                    ps = pjp.tile([128, 512], f32, tag="pj")
                    sl = slice(jbb * 1024 + h * 512, jbb * 1024 + (h + 1) * 512)
                    for dc in range(DC):
                        nc.tensor.matmul(ps[:], wq_all[:, dc, qm * 128:(qm + 1) * 128],
                                         xt_all[:, dc, sl],
                                         start=(dc == 0), stop=(dc == DC - 1))
                    nc.vector.tensor_copy(qpair[:, qm, sl], ps[:])
                return run, 8192

            def o_piece(sm, pc, split_dma=False):
                def run():
                    ps = pjp.tile([128, 512], f32, tag="pj")
                    for cc in range(4):
                        nc.tensor.matmul(ps[:], at[:, cc, sm * 128:(sm + 1) * 128],
                                         wo_all[:, cc, pc * 512:(pc + 1) * 512],
                                         start=(cc == 0), stop=(cc == 3))
                    o_sb = ev.tile([128, 512], f32, tag="o_sb")
                    nc.vector.tensor_copy(o_sb[:], ps[:])
                    rs = slice(sm * 128, (sm + 1) * 128)
                    if split_dma:
                        for u in range(4):
                            cs = slice(pc * 512 + u * 128, pc * 512 + (u + 1) * 128)
                            nc.sync.dma_start(out=out[rs, cs], in_=o_sb[:, u * 128:(u + 1) * 128])
                    else:
                        nc.sync.dma_start(out=out[rs, pc * 512:(pc + 1) * 512], in_=o_sb[:])
                return run, 2048

            work = []
            budget = [0]

            def fill(cycles):
                budget[0] += cycles
                while work and budget[0] > 0:
                    run, cost = work.pop(0)
                    run()
                    budget[0] -= cost

            # prologue: v chunks 0-2 + q halves for pairs 0,1 run eagerly so
            # the first attention pair has everything it needs
            for sc in range(3):
                v_chunk(sc)[0]()
            for sc in range(3, SC):
                work.append(v_chunk(sc))
            for qm, h in [(0, 0), (0, 1), (1, 0), (1, 1)]:
                q_half(qm, 0, h)[0]()
            for qm in (2, 3):
                work.append(q_half(qm, 0, 0))
                work.append(q_half(qm, 0, 1))
            for qm in range(4):
                work.append(q_half(qm, 1, 0))
                work.append(q_half(qm, 1, 1))

            # packed AV accumulator slots: 18 x [128, 65] f32 in 3 PSUM banks
            def o_slot(o_all, s, lo, hi):
                b, i = s // 6, s % 6
                return o_all[:, b, 85 * i + lo:85 * i + hi]

            # ------------- fused attention, per (jb, qm) head pair ----------
            pend_nt = []          # deferred normalize+transpose units

            def do_av(o_all, kc, p_A, p_B, kv):
                for h2, p in ((0, p_A), (1, p_B)):
                    for qt in range(8):
                        dst = o_slot(o_all, h2 * 8 + qt, 0, 65)
                        nc.tensor.matmul(dst, p[:, qt * 128:(qt + 1) * 128],
                                         vv[:, kc, kv * 65:kv * 65 + 65],
                                         start=(kc == 0), stop=(kc == SC - 1))

            for jb in range(2):
                qsl = slice(jb * 1024, (jb + 1) * 1024)
                for qm in range(4):
                    kv = qm // 2
                    o_all = oap.tile([128, 3, 512], f32, tag="o_all")
                    prev = None
                    for kc in range(SC):
                        ksl = slice(kc * 128, (kc + 1) * 128)
                        ps_A = sps.tile([128, 1024], f32, tag="s_ps")
                        mmacc(ps_A, ktd[0:64, kv, ksl],
                              qpair[0:64, qm, qsl], 1024, True, True)
                        p_A = ppool.tile([128, 1024], bf16, tag="p_A")
                        nc.scalar.activation(p_A[:], ps_A[:], Exp, scale=SCALE)
                        ps_B = sps.tile([128, 1024], f32, tag="s_ps")
                        mmacc(ps_B, ktd[64:128, kv, ksl],
                              qpair[64:128, qm, qsl], 1024, True, True)
                        p_B = ppool.tile([128, 1024], bf16, tag="p_B")
                        nc.scalar.activation(p_B[:], ps_B[:], Exp, scale=SCALE)
                        fill(2048)
                        if prev is not None:
                            do_av(o_all, *prev, kv)
                        prev = (kc, p_A, p_B)
                    do_av(o_all, *prev, kv)

                    # normalize (per-partition denominators) + PE transpose
                    # back to at^T[d, q]; filler between transposes keeps the
                    # PE streaming while DVE/ACT chew the normalize chain
                    for qt in range(8):
                        at_n2 = ev.tile([128, 128], bf16, tag="at_n2")
                        for h2 in range(2):
                            num = o_slot(o_all, h2 * 8 + qt, 0, 64)
                            den = o_slot(o_all, h2 * 8 + qt, 64, 65)
                            rden = dpool.tile([128, 1], f32, tag="rden")
                            nc.vector.reciprocal(rden[:], den)
                            nc.scalar.activation(at_n2[:, h2 * 64:(h2 + 1) * 64],
                                                 num, Copy, scale=rden[:])
                        tp = pjp.tile([128, 128], bf16, tag="pj")
                        nc.tensor.matmul(tp[:], at_n2[:], ident[:],
                                         is_transpose=True)
                        nc.vector.tensor_copy(
                            at[:, qm, jb * 1024 + qt * 128:jb * 1024 + (qt + 1) * 128],
                            tp[:])
                        fill(512)

                # after all 4 pairs of this jb, at[:, :, jb half] is complete:
                # queue the o_proj pieces for its seq tiles
                for sm in range(jb * 8, (jb + 1) * 8):
                    for pc in range(4):
                        work.append(o_piece(sm, pc, split_dma=(sm == 15)))

            # epilogue: drain remaining o_proj work (last tile uses split
            # DMAs so the final transfer tail is short)
            while work:
                work.pop(0)[0]()

    nc.compile()
    return nc


def _get_nc():
    if "nc" not in _CACHE:
        _CACHE["nc"] = _build()
    return _CACHE["nc"]


def kernel(x, wq, wk, wv, wo):
    from concourse.bass_utils import run_bass_kernel_spmd

    bf16 = ml_dtypes.bfloat16
    nc = _get_nc()

    in_maps = []
    for core in range(8):
        b, g = core // 4, core % 4
        in_maps.append({
            "xt": np.ascontiguousarray(np.asarray(x)[b].T).astype(bf16),
            "wq": np.ascontiguousarray(np.asarray(wq)[:, g * QC:(g + 1) * QC]).astype(bf16),
            "wk": np.ascontiguousarray(np.asarray(wk)[:, g * KVC:(g + 1) * KVC]).astype(bf16),
            "wv": np.ascontiguousarray(np.asarray(wv)[:, g * KVC:(g + 1) * KVC]).astype(bf16),
            "wo": np.ascontiguousarray(np.asarray(wo)[g * QC:(g + 1) * QC, :]).astype(bf16),
        })

    res = run_bass_kernel_spmd(nc, in_maps, core_ids=list(range(8)))
    outs = [res.results[c]["out"] for c in range(8)]
    full = np.empty((2, S, D), np.float32)
    full[0] = outs[0] + outs[1] + outs[2] + outs[3]
    full[1] = outs[4] + outs[5] + outs[6] + outs[7]
    return full


# revision 10
# speedup vs baseline: 1.0812x; 1.0812x over previous
"""LlamaAttention (GQA, no mask) on 8 Trainium2 NeuronCores.

Sharding: 8 cores = 2 (batch) x 4 (head groups of 8 heads / 2 KV heads).
Per core (bf16 compute, fp32 accumulation):
  qT  = (x_b @ wq_g)^T            [512, 2048]   (head dims on partitions)
  kTd = (x_b @ wk_g)^T duplicated [128, 2, 2048]
  v   = x_b @ wv_g (+ ones col)   [2048, 2, 65]
  per head pair: sT[k,q] matmuls -> exp on ACT -> flipped AV matmuls
    out[q-tile 128, 65] (full-M, half the PE streaming of the [65, q]
    orientation), accumulated in a packed 3-bank PSUM tile; per-partition
    reciprocal + ACT scale-mul normalize; PE transpose (identity matmul)
    restores at^T[d, q] for o_proj.
  out_partial = at @ wo_g         [2048, 2048] fp32
Host sums the 4 head-group partials per batch.
v/q/o projection matmuls are emitted as ~2048-cycle filler quanta inside
the attention kc loop (dedicated 1-bank PSUM buffer) so the PE array keeps
streaming while ACT computes exp.
"""

import numpy as np
import ml_dtypes

S = 2048          # sequence length
D = 2048          # model dim
HD = 64           # head dim
GH = 8            # heads per core
QC = GH * HD      # 512 q cols per core
KVC = 128         # kv cols per core (2 kv heads)
DC = D // 128     # 16 contraction chunks
SC = S // 128     # 16 seq chunks
SCALE = HD ** -0.5

_CACHE = {}


def _build():
    import concourse.bass as bass
    import concourse.mybir as mybir
    import concourse.tile as tile
    from concourse import bacc, masks

    f32 = mybir.dt.float32
    bf16 = mybir.dt.bfloat16
    Exp = mybir.ActivationFunctionType.Exp
    Copy = mybir.ActivationFunctionType.Copy

    nc = bacc.Bacc("TRN2", target_bir_lowering=False, debug=False, num_devices=8)

    xt = nc.dram_tensor("xt", [D, S], bf16, kind="ExternalInput").ap()
    wq = nc.dram_tensor("wq", [D, QC], bf16, kind="ExternalInput").ap()
    wk = nc.dram_tensor("wk", [D, KVC], bf16, kind="ExternalInput").ap()
    wv = nc.dram_tensor("wv", [D, KVC], bf16, kind="ExternalInput").ap()
    wo = nc.dram_tensor("wo", [QC, D], bf16, kind="ExternalInput").ap()
    out = nc.dram_tensor("out", [S, D], f32, kind="ExternalOutput").ap()

    with tile.TileContext(nc) as tc:
        with tc.tile_pool(name="const", bufs=1) as const, \
             tc.tile_pool(name="sps", bufs=2, space="PSUM") as sps, \
             tc.tile_pool(name="pjp", bufs=1, space="PSUM") as pjp, \
             tc.tile_pool(name="oap", bufs=1, space="PSUM") as oap, \
             tc.tile_pool(name="ev", bufs=2) as ev, \
             tc.tile_pool(name="ppool", bufs=3) as ppool, \
             tc.tile_pool(name="dpool", bufs=2) as dpool:

            # resident inputs, partition-chunked layouts (small weights first,
            # xt per-chunk so the k/v projections can start early)
            wk_all = const.tile([128, DC, KVC], bf16, tag="wk_all")
            nc.sync.dma_start(out=wk_all[:], in_=wk.rearrange("(c p) n -> p c n", p=128))
            wv_all = const.tile([128, DC, KVC], bf16, tag="wv_all")
            nc.sync.dma_start(out=wv_all[:], in_=wv.rearrange("(c p) n -> p c n", p=128))
            xt_all = const.tile([128, DC, S], bf16, tag="xt_all")
            xt_re = xt.rearrange("(c p) s -> p c s", p=128)
            for dc in range(DC):
                nc.sync.dma_start(out=xt_all[:, dc, :], in_=xt_re[:, dc, :])
            wq_all = const.tile([128, DC, QC], bf16, tag="wq_all")
            nc.sync.dma_start(out=wq_all[:], in_=wq.rearrange("(c p) n -> p c n", p=128))
            wo_all = const.tile([128, QC // 128, D], bf16, tag="wo_all")
            nc.sync.dma_start(out=wo_all[:], in_=wo.rearrange("(c p) n -> p c n", p=128))

            ident = const.tile([128, 128], bf16, tag="ident")
            masks.make_identity(nc, ident[:])

            # persistent intermediates
            qpair = const.tile([128, 4, S], bf16, tag="qpair")     # q^T
            ktd = const.tile([128, 2, S], bf16, tag="ktd")         # k^T dup per kv head
            vv = const.tile([128, SC, 130], bf16, tag="vv")        # v (+ones cols)
            at = const.tile([128, 4, S], bf16, tag="at")           # attn out^T

            nc.vector.memset(vv[:, :, 64:65], 1.0)
            nc.vector.memset(vv[:, :, 129:130], 1.0)

            def mmacc(out_t, lhsT, rhs, width, start, stop):
                # moving-operand ISA limit is 512: split wide matmuls
                for o in range(0, width, 512):
                    nc.tensor.matmul(out_t[:, o:o + 512], lhsT,
                                     rhs[:, o:o + 512], start=start, stop=stop)

            # ---------------- k projection (needed before any scores) -------
            for nb in range(2):
                ps = sps.tile([128, 1024], f32, tag="s_ps")
                for dc in range(DC):
                    mmacc(ps, wk_all[:, dc, :],
                          xt_all[:, dc, nb * 1024:(nb + 1) * 1024], 1024,
                          (dc == 0), (dc == DC - 1))
                kt_sb = ev.tile([128, 1024], bf16, tag="kt_sb")
                nc.vector.tensor_copy(kt_sb[:], ps[:])
                sl = slice(nb * 1024, (nb + 1) * 1024)
                nc.sync.dma_start(out=ktd[0:64, 0, sl], in_=kt_sb[0:64, :])
                nc.sync.dma_start(out=ktd[64:128, 0, sl], in_=kt_sb[0:64, :])
                nc.sync.dma_start(out=ktd[0:64, 1, sl], in_=kt_sb[64:128, :])
                nc.sync.dma_start(out=ktd[64:128, 1, sl], in_=kt_sb[64:128, :])

            # ---------------- filler work units (~2048 PE cycles each) ------
            def v_chunk(sc):
                def run():
                    ps = pjp.tile([128, 512], f32, tag="pj")
                    for dc in range(DC):
                        nc.tensor.matmul(ps[:, 0:KVC],
                                         xt_all[:, dc, sc * 128:(sc + 1) * 128],
                                         wv_all[:, dc, :],
                                         start=(dc == 0), stop=(dc == DC - 1))
                    nc.vector.tensor_copy(vv[:, sc, 0:64], ps[:, 0:64])
                    nc.vector.tensor_copy(vv[:, sc, 65:129], ps[:, 64:128])
                return run, 2048

            def q_half(qm, jbb, h):
                def run():
                    ps = pjp.tile([128, 512], f32, tag="pj")
                    sl = slice(jbb * 1024 + h * 512, jbb * 1024 + (h + 1) * 512)
                    for dc in range(DC):
                        nc.tensor.matmul(ps[:], wq_all[:, dc, qm * 128:(qm + 1) * 128],
                                         xt_all[:, dc, sl],
                                         start=(dc == 0), stop=(dc == DC - 1))
                    nc.vector.tensor_copy(qpair[:, qm, sl], ps[:])
                return run, 8192

            def o_piece(sm, pc, split_dma=False):
                def run():
                    ps = pjp.tile([128, 512], f32, tag="pj")
                    for cc in range(4):
                        nc.tensor.matmul(ps[:], at[:, cc, sm * 128:(sm + 1) * 128],
                                         wo_all[:, cc, pc * 512:(pc + 1) * 512],
                                         start=(cc == 0), stop=(cc == 3))
                    o_sb = ev.tile([128, 512], f32, tag="o_sb")
                    nc.vector.tensor_copy(o_sb[:], ps[:])
                    rs = slice(sm * 128, (sm + 1) * 128)
                    if split_dma:
                        for u in range(4):
                            cs = slice(pc * 512 + u * 128, pc * 512 + (u + 1) * 128)
                            nc.sync.dma_start(out=out[rs, cs], in_=o_sb[:, u * 128:(u + 1) * 128])
                    else:
                        nc.sync.dma_start(out=out[rs, pc * 512:(pc + 1) * 512], in_=o_sb[:])
                return run, 2048

            work = []
            budget = [0]
            releases = {}         # pair index -> list of units released there

            def fill(cycles):
                budget[0] += cycles
                while work and budget[0] > 0:
                    run, cost = work.pop(0)
                    run()
                    budget[0] -= cost

            # prologue: v chunks 0-2 + q halves for pairs 0,1 run eagerly so
            # the first attention pair has everything it needs. Later work is
            # release-gated to its pair so jb=1 keeps PE filler too.
            for sc in range(3):
                v_chunk(sc)[0]()
            for sc in range(3, SC):
                work.append(v_chunk(sc))
            for qm, h in [(0, 0), (0, 1), (1, 0), (1, 1)]:
                q_half(qm, 0, h)[0]()
            work.append(q_half(2, 0, 0))
            work.append(q_half(2, 0, 1))
            releases[1] = [q_half(3, 0, 0), q_half(3, 0, 1)]
            releases[2] = [q_half(0, 1, 0), q_half(0, 1, 1)]
            releases[3] = [q_half(1, 1, 0), q_half(1, 1, 1)]
            releases[4] = [q_half(2, 1, 0), q_half(2, 1, 1)]
            releases[5] = [q_half(3, 1, 0), q_half(3, 1, 1)]

            # packed AV accumulator slots: 18 x [128, 65] f32 in 3 PSUM banks
            def o_slot(o_all, s, lo, hi):
                b, i = s // 6, s % 6
                return o_all[:, b, 85 * i + lo:85 * i + hi]

            # ------------- fused attention, per (jb, qm) head pair ----------
            def transpose_out(at_n2, qm, jbb, qt):
                tp = pjp.tile([128, 128], bf16, tag="pj")
                nc.tensor.matmul(tp[:], at_n2[:], ident[:], is_transpose=True)
                nc.vector.tensor_copy(
                    at[:, qm, jbb * 1024 + qt * 128:jbb * 1024 + (qt + 1) * 128],
                    tp[:])

            def do_av(o_all, kc, p_A, p_B, kv):
                for h2, p in ((0, p_A), (1, p_B)):
                    for qt in range(8):
                        dst = o_slot(o_all, h2 * 8 + qt, 0, 65)
                        nc.tensor.matmul(dst, p[:, qt * 128:(qt + 1) * 128],
                                         vv[:, kc, kv * 65:kv * 65 + 65],
                                         start=(kc == 0), stop=(kc == SC - 1))

            for jb in range(2):
                qsl = slice(jb * 1024, (jb + 1) * 1024)
                for qm in range(4):
                    pair = jb * 4 + qm
                    work[:0] = releases.get(pair, [])
                    kv = qm // 2
                    o_all = oap.tile([128, 3, 512], f32, tag="o_all")
                    prev = None
                    for kc in range(SC):
                        # AV for kc-1 + filler go first so the scores matmul
                        # (which waits on exp freeing its psum slot) never
                        # blocks independent PE work behind it
                        if prev is not None:
                            do_av(o_all, *prev, kv)
                        fill(2048)
                        ksl = slice(kc * 128, (kc + 1) * 128)
                        ps_A = sps.tile([128, 1024], f32, tag="s_ps")
                        mmacc(ps_A, ktd[0:64, kv, ksl],
                              qpair[0:64, qm, qsl], 1024, True, True)
                        p_A = ppool.tile([128, 1024], bf16, tag="p_A")
                        nc.scalar.activation(p_A[:], ps_A[:], Exp, scale=SCALE)
                        ps_B = sps.tile([128, 1024], f32, tag="s_ps")
                        mmacc(ps_B, ktd[64:128, kv, ksl],
                              qpair[64:128, qm, qsl], 1024, True, True)
                        p_B = ppool.tile([128, 1024], bf16, tag="p_B")
                        nc.scalar.activation(p_B[:], ps_B[:], Exp, scale=SCALE)
                        prev = (kc, p_A, p_B)
                    do_av(o_all, *prev, kv)

                    # normalize (per-partition denominators, DVE) + PE
                    # transpose back to at^T[d, q], software-pipelined: the
                    # DVE chain for qt runs while PE transposes qt-1
                    at_n2s = []
                    for qt in range(8):
                        at_n2 = ev.tile([128, 128], bf16, tag="at_n2")
                        for h2 in range(2):
                            num = o_slot(o_all, h2 * 8 + qt, 0, 64)
                            den = o_slot(o_all, h2 * 8 + qt, 64, 65)
                            rden = dpool.tile([128, 1], f32, tag="rden")
                            nc.vector.reciprocal(rden[:], den)
                            nc.vector.tensor_scalar_mul(
                                at_n2[:, h2 * 64:(h2 + 1) * 64], num, rden[:])
                        at_n2s.append(at_n2)
                        if qt >= 1:
                            transpose_out(at_n2s[qt - 1], qm, jb, qt - 1)
                            fill(512)
                    transpose_out(at_n2s[7], qm, jb, 7)
                    fill(512)

                # after jb=0's 4 pairs, at[:, :, 0:1024] is complete: queue
                # o_proj pieces for its seq tiles as jb=1 filler (jb=1's own
                # seq tiles run in the epilogue instead)
                if jb == 0:
                    for sm in range(8):
                        for pc in range(4):
                            work.append(o_piece(sm, pc))

            # epilogue: first drain any leftover queued work, then run jb=1's
            # o_proj on the freed scores pool with wide tiles (no
            # single-buffer serialization); the last seq tile uses split
            # DMAs so the final transfer tail is short
            while work:
                work.pop(0)[0]()
            for sm in range(8, 16):
                for nb in range(2):
                    ps = sps.tile([128, 1024], f32, tag="s_ps")
                    for cc in range(4):
                        mmacc(ps, at[:, cc, sm * 128:(sm + 1) * 128],
                              wo_all[:, cc, nb * 1024:(nb + 1) * 1024], 1024,
                              (cc == 0), (cc == 3))
                    o_sb = ev.tile([128, 1024], f32, tag="o_sb2")
                    nc.vector.tensor_copy(o_sb[:], ps[:])
                    rs = slice(sm * 128, (sm + 1) * 128)
                    nsplit = 4 if sm == 15 else 1
                    w = 1024 // nsplit
                    for u in range(nsplit):
                        cs = slice(nb * 1024 + u * w, nb * 1024 + (u + 1) * w)
                        nc.sync.dma_start(out=out[rs, cs],
                                          in_=o_sb[:, u * w:(u + 1) * w])

    nc.compile()
    return nc


def _get_nc():
    if "nc" not in _CACHE:
        _CACHE["nc"] = _build()
    return _CACHE["nc"]


def kernel(x, wq, wk, wv, wo):
    from concourse.bass_utils import run_bass_kernel_spmd

    bf16 = ml_dtypes.bfloat16
    nc = _get_nc()

    in_maps = []
    for core in range(8):
        b, g = core // 4, core % 4
        in_maps.append({
            "xt": np.ascontiguousarray(np.asarray(x)[b].T).astype(bf16),
            "wq": np.ascontiguousarray(np.asarray(wq)[:, g * QC:(g + 1) * QC]).astype(bf16),
            "wk": np.ascontiguousarray(np.asarray(wk)[:, g * KVC:(g + 1) * KVC]).astype(bf16),
            "wv": np.ascontiguousarray(np.asarray(wv)[:, g * KVC:(g + 1) * KVC]).astype(bf16),
            "wo": np.ascontiguousarray(np.asarray(wo)[g * QC:(g + 1) * QC, :]).astype(bf16),
        })

    res = run_bass_kernel_spmd(nc, in_maps, core_ids=list(range(8)))
    outs = [res.results[c]["out"] for c in range(8)]
    full = np.empty((2, S, D), np.float32)
    full[0] = outs[0] + outs[1] + outs[2] + outs[3]
    full[1] = outs[4] + outs[5] + outs[6] + outs[7]
    return full


# revision 16
# speedup vs baseline: 1.1710x; 1.0830x over previous
"""LlamaAttention (GQA, no mask) on 8 Trainium2 NeuronCores.

Sharding: 8 cores = 2 (batch) x 4 (head groups of 8 heads / 2 KV heads).
Per core (bf16 compute, fp32 accumulation):
  qT  = (x_b @ wq_g)^T            [512, 2048]   (head dims on partitions)
  kTd = (x_b @ wk_g)^T duplicated [128, 2, 2048]
  v   = x_b @ wv_g (+ ones col)   [2048, 2, 65]
  per head pair: sT[k,q] matmuls -> exp on ACT -> flipped AV matmuls
    out[q-tile 128, 65] (full-M, half the PE streaming of the [65, q]
    orientation), accumulated in a packed 3-bank PSUM tile; per-partition
    reciprocal + ACT scale-mul normalize; PE transpose (identity matmul)
    restores at^T[d, q] for o_proj.
  out_partial = at @ wo_g         [2048, 2048] fp32
Host sums the 4 head-group partials per batch.
v/q/o projection matmuls are emitted as ~2048-cycle filler quanta inside
the attention kc loop (dedicated 1-bank PSUM buffer) so the PE array keeps
streaming while ACT computes exp.
"""

import numpy as np
import ml_dtypes

S = 2048          # sequence length
D = 2048          # model dim
HD = 64           # head dim
GH = 8            # heads per core
QC = GH * HD      # 512 q cols per core
KVC = 128         # kv cols per core (2 kv heads)
DC = D // 128     # 16 contraction chunks
SC = S // 128     # 16 seq chunks
SCALE = HD ** -0.5

_CACHE = {}


def _build():
    import concourse.bass as bass
    import concourse.mybir as mybir
    import concourse.tile as tile
    from concourse import bacc, masks

    f32 = mybir.dt.float32
    bf16 = mybir.dt.bfloat16
    Exp = mybir.ActivationFunctionType.Exp
    Copy = mybir.ActivationFunctionType.Copy

    nc = bacc.Bacc("TRN2", target_bir_lowering=False, debug=False, num_devices=8)

    xt = nc.dram_tensor("xt", [D, S], bf16, kind="ExternalInput").ap()
    wq = nc.dram_tensor("wq", [D, QC], bf16, kind="ExternalInput").ap()
    wk = nc.dram_tensor("wk", [D, KVC], bf16, kind="ExternalInput").ap()
    wv = nc.dram_tensor("wv", [D, KVC], bf16, kind="ExternalInput").ap()
    wo = nc.dram_tensor("wo", [QC, D], bf16, kind="ExternalInput").ap()
    out = nc.dram_tensor("out", [S, D], f32, kind="ExternalOutput").ap()

    with tile.TileContext(nc) as tc:
        with tc.tile_pool(name="const", bufs=1) as const, \
             tc.tile_pool(name="sps", bufs=2, space="PSUM") as sps, \
             tc.tile_pool(name="pjp", bufs=1, space="PSUM") as pjp, \
             tc.tile_pool(name="oap", bufs=1, space="PSUM") as oap, \
             tc.tile_pool(name="ev", bufs=2) as ev, \
             tc.tile_pool(name="ppool", bufs=3) as ppool, \
             tc.tile_pool(name="dpool", bufs=2) as dpool:

            # resident inputs, partition-chunked layouts (small weights first,
            # xt per-chunk so the k/v projections can start early)
            wk_all = const.tile([128, DC, KVC], bf16, tag="wk_all")
            nc.sync.dma_start(out=wk_all[:], in_=wk.rearrange("(c p) n -> p c n", p=128))
            wv_all = const.tile([128, DC, KVC], bf16, tag="wv_all")
            nc.sync.dma_start(out=wv_all[:], in_=wv.rearrange("(c p) n -> p c n", p=128))
            xt_all = const.tile([128, DC, S], bf16, tag="xt_all")
            xt_re = xt.rearrange("(c p) s -> p c s", p=128)
            for dc in range(DC):
                nc.sync.dma_start(out=xt_all[:, dc, :], in_=xt_re[:, dc, :])
            wq_all = const.tile([128, DC, QC], bf16, tag="wq_all")
            nc.sync.dma_start(out=wq_all[:], in_=wq.rearrange("(c p) n -> p c n", p=128))
            wo_all = const.tile([128, QC // 128, D], bf16, tag="wo_all")
            nc.sync.dma_start(out=wo_all[:], in_=wo.rearrange("(c p) n -> p c n", p=128))

            ident = const.tile([128, 128], bf16, tag="ident")
            masks.make_identity(nc, ident[:])

            # persistent intermediates
            qpair = const.tile([128, 4, S], bf16, tag="qpair")     # q^T
            ktd = const.tile([128, 2, S], bf16, tag="ktd")         # k^T dup per kv head
            vv = const.tile([128, SC, 130], bf16, tag="vv")        # v (+ones cols)
            at = const.tile([128, 4, S], bf16, tag="at")           # attn out^T

            nc.vector.memset(vv[:, :, 64:65], 1.0)
            nc.vector.memset(vv[:, :, 129:130], 1.0)

            def mmacc(out_t, lhsT, rhs, width, start, stop):
                # moving-operand ISA limit is 512: split wide matmuls
                for o in range(0, width, 512):
                    nc.tensor.matmul(out_t[:, o:o + 512], lhsT,
                                     rhs[:, o:o + 512], start=start, stop=stop)

            # ---------------- k projection (needed before any scores) -------
            for nb in range(2):
                ps = sps.tile([128, 1024], f32, tag="s_ps")
                for dc in range(DC):
                    mmacc(ps, wk_all[:, dc, :],
                          xt_all[:, dc, nb * 1024:(nb + 1) * 1024], 1024,
                          (dc == 0), (dc == DC - 1))
                kt_sb = ev.tile([128, 1024], bf16, tag="kt_sb")
                nc.vector.tensor_copy(kt_sb[:], ps[:])
                sl = slice(nb * 1024, (nb + 1) * 1024)
                nc.sync.dma_start(out=ktd[0:64, 0, sl], in_=kt_sb[0:64, :])
                nc.sync.dma_start(out=ktd[64:128, 0, sl], in_=kt_sb[0:64, :])
                nc.sync.dma_start(out=ktd[0:64, 1, sl], in_=kt_sb[64:128, :])
                nc.sync.dma_start(out=ktd[64:128, 1, sl], in_=kt_sb[64:128, :])

            # ---------------- filler work units (~2048 PE cycles each) ------
            def v_chunk(sc):
                def run():
                    ps = pjp.tile([128, 512], f32, tag="pj")
                    for dc in range(DC):
                        nc.tensor.matmul(ps[:, 0:KVC],
                                         xt_all[:, dc, sc * 128:(sc + 1) * 128],
                                         wv_all[:, dc, :],
                                         start=(dc == 0), stop=(dc == DC - 1))
                    yield 2048
                    nc.vector.tensor_copy(vv[:, sc, 0:64], ps[:, 0:64])
                    nc.vector.tensor_copy(vv[:, sc, 65:129], ps[:, 64:128])
                return run

            def q_half(qm, jbb, h):
                # generator: emits in 4-dc quanta (~2048 PE cycles each) so
                # the filler spreads across kc slots instead of blobbing
                def run():
                    ps = pjp.tile([128, 512], f32, tag="pj")
                    sl = slice(jbb * 1024 + h * 512, jbb * 1024 + (h + 1) * 512)
                    for dq in range(0, DC, 4):
                        for dc in range(dq, dq + 4):
                            nc.tensor.matmul(ps[:], wq_all[:, dc, qm * 128:(qm + 1) * 128],
                                             xt_all[:, dc, sl],
                                             start=(dc == 0), stop=(dc == DC - 1))
                        yield 2048
                    nc.vector.tensor_copy(qpair[:, qm, sl], ps[:])
                return run

            def o_piece(sm, pc):
                def run():
                    ps = pjp.tile([128, 512], f32, tag="pj")
                    for cc in range(4):
                        nc.tensor.matmul(ps[:], at[:, cc, sm * 128:(sm + 1) * 128],
                                         wo_all[:, cc, pc * 512:(pc + 1) * 512],
                                         start=(cc == 0), stop=(cc == 3))
                    yield 2048
                    o_sb = ev.tile([128, 512], f32, tag="o_sb")
                    nc.vector.tensor_copy(o_sb[:], ps[:])
                    rs = slice(sm * 128, (sm + 1) * 128)
                    nc.sync.dma_start(out=out[rs, pc * 512:(pc + 1) * 512], in_=o_sb[:])
                return run

            work = []
            budget = [0]
            current = [None]      # in-flight filler generator
            releases = {}         # pair index -> list of units released there

            def fill(cycles):
                budget[0] += cycles
                while budget[0] > 0:
                    if current[0] is None:
                        if not work:
                            return
                        current[0] = work.pop(0)()
                    try:
                        budget[0] -= next(current[0])
                    except StopIteration:
                        current[0] = None

            def drain_filler():
                if current[0] is not None:
                    for _ in current[0]:
                        pass
                    current[0] = None
                while work:
                    for _ in work.pop(0)():
                        pass

            # prologue: v chunks 0-2 + q halves for pairs 0,1 run eagerly so
            # the first attention pair has everything it needs. Later work is
            # release-gated to its pair so jb=1 keeps PE filler too.
            for sc in range(3):
                for _ in v_chunk(sc)():
                    pass
            for sc in range(3, SC):
                work.append(v_chunk(sc))
            for qm, h in [(0, 0), (0, 1), (1, 0), (1, 1)]:
                for _ in q_half(qm, 0, h)():
                    pass
            work.append(q_half(2, 0, 0))
            work.append(q_half(2, 0, 1))
            releases[1] = [q_half(3, 0, 0), q_half(3, 0, 1)]
            releases[2] = [q_half(0, 1, 0), q_half(0, 1, 1)]
            releases[3] = [q_half(1, 1, 0), q_half(1, 1, 1)]
            releases[4] = [q_half(2, 1, 0), q_half(2, 1, 1)]
            releases[5] = [q_half(3, 1, 0), q_half(3, 1, 1)]

            # packed AV accumulator slots: 18 x [128, 65] f32 in 3 PSUM banks
            def o_slot(o_all, s, lo, hi):
                b, i = s // 6, s % 6
                return o_all[:, b, 85 * i + lo:85 * i + hi]

            # ------------- fused attention, per (jb, qm) head pair ----------
            def transpose_out(at_n2, qm, jbb, qt):
                tp = pjp.tile([128, 128], bf16, tag="pj")
                nc.tensor.matmul(tp[:], at_n2[:], ident[:], is_transpose=True)
                nc.vector.tensor_copy(
                    at[:, qm, jbb * 1024 + qt * 128:jbb * 1024 + (qt + 1) * 128],
                    tp[:])

            def do_av(o_all, kc, p_A, p_B, kv):
                for h2, p in ((0, p_A), (1, p_B)):
                    for qt in range(8):
                        dst = o_slot(o_all, h2 * 8 + qt, 0, 65)
                        nc.tensor.matmul(dst, p[:, qt * 128:(qt + 1) * 128],
                                         vv[:, kc, kv * 65:kv * 65 + 65],
                                         start=(kc == 0), stop=(kc == SC - 1))

            for jb in range(2):
                qsl = slice(jb * 1024, (jb + 1) * 1024)
                for qm in range(4):
                    pair = jb * 4 + qm
                    work[:0] = releases.get(pair, [])
                    kv = qm // 2
                    o_all = oap.tile([128, 3, 512], f32, tag="o_all")
                    prev = None
                    for kc in range(SC):
                        # AV for kc-1 + filler go first so the scores matmul
                        # (which waits on exp freeing its psum slot) never
                        # blocks independent PE work behind it
                        if prev is not None:
                            do_av(o_all, *prev, kv)
                        fill(2048)
                        ksl = slice(kc * 128, (kc + 1) * 128)
                        ps_A = sps.tile([128, 1024], f32, tag="s_ps")
                        mmacc(ps_A, ktd[0:64, kv, ksl],
                              qpair[0:64, qm, qsl], 1024, True, True)
                        p_A = ppool.tile([128, 1024], bf16, tag="p_A")
                        nc.scalar.activation(p_A[:], ps_A[:], Exp, scale=SCALE)
                        ps_B = sps.tile([128, 1024], f32, tag="s_ps")
                        mmacc(ps_B, ktd[64:128, kv, ksl],
                              qpair[64:128, qm, qsl], 1024, True, True)
                        p_B = ppool.tile([128, 1024], bf16, tag="p_B")
                        nc.scalar.activation(p_B[:], ps_B[:], Exp, scale=SCALE)
                        prev = (kc, p_A, p_B)
                    do_av(o_all, *prev, kv)

                    # bulk-copy the AV accumulators to SBUF immediately so
                    # the PSUM banks free up for the next pair's AV, then
                    # normalize (per-partition denominators, DVE) + PE
                    # transpose back to at^T[d, q]
                    o_st = ev.tile([128, 3, 512], f32, tag="o_st")
                    nc.vector.tensor_copy(o_st[:], o_all[:])
                    at_n2s = []
                    for qt in range(8):
                        at_n2 = ev.tile([128, 128], bf16, tag="at_n2")
                        for h2 in range(2):
                            num = o_slot(o_st, h2 * 8 + qt, 0, 64)
                            den = o_slot(o_st, h2 * 8 + qt, 64, 65)
                            rden = dpool.tile([128, 1], f32, tag="rden")
                            nc.vector.reciprocal(rden[:], den)
                            nc.vector.tensor_scalar_mul(
                                at_n2[:, h2 * 64:(h2 + 1) * 64], num, rden[:])
                        at_n2s.append(at_n2)
                        if qt >= 1:
                            transpose_out(at_n2s[qt - 1], qm, jb, qt - 1)
                            fill(512)
                    transpose_out(at_n2s[7], qm, jb, 7)
                    fill(512)

                # after jb=0's 4 pairs, at[:, :, 0:1024] is complete: queue
                # o_proj pieces for its seq tiles as jb=1 filler (jb=1's own
                # seq tiles run in the epilogue instead)
                if jb == 0:
                    for sm in range(8):
                        for pc in range(4):
                            work.append(o_piece(sm, pc))

            # epilogue: first drain any leftover queued work, then run jb=1's
            # o_proj on the freed scores pool with wide tiles (no
            # single-buffer serialization); the last seq tile uses split
            # DMAs so the final transfer tail is short
            drain_filler()
            for sm in range(8, 16):
                for nb in range(2):
                    ps = sps.tile([128, 1024], f32, tag="s_ps")
                    for cc in range(4):
                        mmacc(ps, at[:, cc, sm * 128:(sm + 1) * 128],
                              wo_all[:, cc, nb * 1024:(nb + 1) * 1024], 1024,
                              (cc == 0), (cc == 3))
                    o_sb = ev.tile([128, 1024], f32, tag="o_sb2")
                    nc.vector.tensor_copy(o_sb[:], ps[:])
                    rs = slice(sm * 128, (sm + 1) * 128)
                    nsplit = 4 if sm == 15 else 1
                    w = 1024 // nsplit
                    for u in range(nsplit):
                        cs = slice(nb * 1024 + u * w, nb * 1024 + (u + 1) * w)
                        nc.sync.dma_start(out=out[rs, cs],
                                          in_=o_sb[:, u * w:(u + 1) * w])

    nc.compile()
    return nc


def _get_nc():
    if "nc" not in _CACHE:
        _CACHE["nc"] = _build()
    return _CACHE["nc"]


def kernel(x, wq, wk, wv, wo):
    from concourse.bass_utils import run_bass_kernel_spmd

    bf16 = ml_dtypes.bfloat16
    nc = _get_nc()

    in_maps = []
    for core in range(8):
        b, g = core // 4, core % 4
        in_maps.append({
            "xt": np.ascontiguousarray(np.asarray(x)[b].T).astype(bf16),
            "wq": np.ascontiguousarray(np.asarray(wq)[:, g * QC:(g + 1) * QC]).astype(bf16),
            "wk": np.ascontiguousarray(np.asarray(wk)[:, g * KVC:(g + 1) * KVC]).astype(bf16),
            "wv": np.ascontiguousarray(np.asarray(wv)[:, g * KVC:(g + 1) * KVC]).astype(bf16),
            "wo": np.ascontiguousarray(np.asarray(wo)[g * QC:(g + 1) * QC, :]).astype(bf16),
        })

    res = run_bass_kernel_spmd(nc, in_maps, core_ids=list(range(8)))
    outs = [res.results[c]["out"] for c in range(8)]
    full = np.empty((2, S, D), np.float32)
    full[0] = outs[0] + outs[1] + outs[2] + outs[3]
    full[1] = outs[4] + outs[5] + outs[6] + outs[7]
    return full
